# revision 10
# baseline (speedup 1.0000x reference)
"""MiniLLM MLA sparse attention on 8 Trainium2 NeuronCores.

Sharding: cyclic over tokens (core c owns tokens t with t % 8 == c), which
makes the causal attention work identical on every core (SPMD-uniform: one
NEFF for all cores). Activations live in [feature, token] layout so matmuls
chain without transposes; all weights are pre-transposed / reordered /
norm-folded on the host. Keys are processed in "gathered" (core-major,
permuted) order; softmax, top-k counting and attn@V are permutation-invariant
over keys, and causality is applied via a host-precomputed mask in the
permuted order.

Numerics: float32r (TF32-like, full PE speed) for all matmuls except the
indexer path, which stays true fp32 so the top-512 selection matches the
reference; the per-row top-k threshold is found by bisection with a fused
compare+count DVE op; softmax skips max-subtraction (scores are O(1)); rows
whose allowed set is empty reproduce the reference's uniform-softmax via a
mean-of-V rank-1 correction.
"""
import numpy as np

import concourse.bass as bass
import concourse.mybir as mybir
import concourse.tile as tile
from concourse.bass_utils import run_bass_kernel_spmd
from wait_split import split_multi_waits

# Problem dims
B, S, H = 1, 2048, 2048
NH = 16
NOPE, ROPE, VD = 128, 64, 128
QD = NOPE + ROPE
QLR, KVLR = 1536, 512
INH, IND, ITOPK = 4, 32, 512
EPS = 1e-6
SCALE = QD ** -0.5
ISCALE = IND ** -0.5

NC = 8            # cores
T = S // NC       # tokens per core (256)
P = 128
NKC = S // P      # key chunks (16)
QT = T // P       # query tiles per core (2)
BISECT_ITERS = 26

HK = H // P       # 16
QK = QLR // P     # 12
KVK = KVLR // P   # 4
G1 = NH * NOPE + ROPE + P   # gather-1 rows: knope + kpe + ik = 2240

F32 = mybir.dt.float32
F32R = mybir.dt.float32r
AF = mybir.ActivationFunctionType
OP = mybir.AluOpType


def _dt(t):
    return np.ascontiguousarray(t, dtype=np.float32)


def host_prep(inputs):
    """Split/reorder/transpose on the host. Returns per-core input maps."""
    x = np.asarray(inputs["x"])[0]                     # [S, H]
    cos = np.asarray(inputs["cos"])                    # [S, ROPE]
    sin = np.asarray(inputs["sin"])
    q_a_w = np.asarray(inputs["q_a_w"])                # [QLR, H]
    q_a_norm_w = np.asarray(inputs["q_a_norm_w"])      # [QLR]
    q_b_w = np.asarray(inputs["q_b_w"])                # [NH*QD, QLR]
    kv_a_w = np.asarray(inputs["kv_a_w"])              # [KVLR+ROPE, H]
    kv_a_norm_w = np.asarray(inputs["kv_a_norm_w"])    # [KVLR]
    kv_b_w = np.asarray(inputs["kv_b_w"])              # [NH*(NOPE+VD), KVLR]
    o_w = np.asarray(inputs["o_w"])                    # [H, NH*VD]
    idx_q_w = np.asarray(inputs["idx_q_w"])            # [INH*IND, H]
    idx_k_w = np.asarray(inputs["idx_k_w"])

    q_b_f = q_b_w * q_a_norm_w[None, :]
    kv_b_f = kv_b_w * kv_a_norm_w[None, :]

    qb3 = q_b_f.reshape(NH, QD, QLR)
    qb_nope = qb3[:, :NOPE, :].reshape(NH * NOPE, QLR)
    qb_rope = qb3[:, NOPE:, :].reshape(NH * ROPE, QLR)
    wqb = _dt(np.concatenate([qb_nope, qb_rope], 0).T)       # [QLR, 3072]

    kb3 = kv_b_f.reshape(NH, NOPE + VD, KVLR)
    wkbk = _dt(kb3[:, :NOPE, :].reshape(NH * NOPE, KVLR).T)  # [KVLR, 2048]
    wkbv = _dt(kb3[:, NOPE:, :].reshape(NH * VD, KVLR).T)    # [KVLR, 2048]

    wqa = _dt(q_a_w.T)                                  # [H, QLR]
    wkva = _dt(kv_a_w.T)                                # [H, KVLR+ROPE]
    wo = _dt(o_w.T)                                     # [NH*VD, H]
    wi = _dt(np.concatenate([idx_q_w, idx_k_w], 0).T)   # [H, 256]

    # rotate_half as matmul: rot = A @ v per 64-block; lhsT = A.T
    A64 = np.zeros((ROPE, ROPE), np.float32)
    A64[:32, 32:] = -np.eye(32, dtype=np.float32)
    A64[32:, :32] = np.eye(32, dtype=np.float32)
    A128 = np.zeros((P, P), np.float32)
    A128[:64, :64] = A64
    A128[64:, 64:] = A64
    rotm = _dt(A128.T)

    ones_col = np.ones((P, 1), np.float32)
    ones_row = np.ones((1, P), np.float32)
    ident = np.eye(P, dtype=np.float32)

    # permuted-order global key index: chunk kc -> core u = kc % 8,
    # half = kc // 8; row i -> global token u + 8*(half*128 + i)
    gkey = np.empty((S,), np.int64)
    for kc in range(NKC):
        u, half = kc % NC, kc // NC
        gkey[kc * P:(kc + 1) * P] = u + NC * (half * P + np.arange(P))

    xT = x.T
    per_core = []
    for c in range(NC):
        toks = np.arange(c, S, NC)
        cosc = cos[toks].T
        sinc = sin[toks].T
        gq = c + NC * np.arange(T)
        causalT = (gkey[:, None] <= gq[None, :]).astype(np.float32)  # [S, T]
        per_core.append({
            "xt": _dt(xT[:, toks]),
            "cos2": _dt(np.concatenate([cosc, cosc], 0)),
            "sin2": _dt(np.concatenate([sinc, sinc], 0)),
            "causal": _dt(causalT),
            "wqa": wqa, "wqb": wqb, "wkva": wkva,
            "wkbk": wkbk, "wkbv": wkbv, "wo": wo, "wi": wi,
            "rotm": rotm, "ones_col": ones_col, "ones_row": ones_row,
            "ident": ident,
        })
    return per_core


def unshard_output(results):
    out = np.empty((B, S, H), np.float32)
    for c in range(NC):
        toks = np.arange(c, S, NC)
        out[0, toks, :] = results[c]["outT"].T
    return out


def build_kernel(with_collective=True):
    nc = bass.Bass(trn_type="TRN2", num_devices=NC)

    for _cv in (EPS, 1e-12):
        _ct = nc.alloc_sbuf_tensor(f"const-f32-{_cv}", [128, 1], F32)
        nc.gpsimd.memset(_ct.ap(), _cv)
        nc.const_aps.aps[(F32, _cv)] = _ct.ap()
    nc.all_engine_barrier()

    def din(name, shape, dtype=F32R):
        return nc.dram_tensor(name, shape, dtype, kind="ExternalInput")

    xt = din("xt", [H, T])
    cos2 = din("cos2", [P, T], F32)
    sin2 = din("sin2", [P, T], F32)
    causal = din("causal", [S, T])
    wqa = din("wqa", [H, QLR])
    wqb = din("wqb", [QLR, NH * QD])
    wkva = din("wkva", [H, KVLR + ROPE])
    wkbk = din("wkbk", [KVLR, NH * NOPE])
    wkbv = din("wkbv", [KVLR, NH * VD])
    wo = din("wo", [NH * VD, H])
    wi = din("wi", [H, 2 * INH * IND], F32)
    rotm = din("rotm", [P, P])
    ones_col = din("ones_col", [P, 1])
    ones_row = din("ones_row", [1, P])
    ident = din("ident", [P, P])

    outT = nc.dram_tensor("outT", [H, T], F32, kind="ExternalOutput")

    # internal DRAM gather buffers
    gin1 = nc.dram_tensor("gin1", [G1, T], F32R)
    gout1 = nc.dram_tensor("gout1", [NC * G1, T], F32R, addr_space="Shared")
    gin2 = nc.dram_tensor("gin2", [T, NH * VD], F32R)
    gout2 = nc.dram_tensor("gout2", [NC * T, NH * VD], F32R,
                           addr_space="Shared")

    with tile.TileContext(nc) as tc:
        with (
            nc.allow_low_precision(reason="float32r is 4-byte; intentional"),
            tc.tile_pool(name="per", bufs=1) as per,
            tc.tile_pool(name="iscg", bufs=1) as iscg,
        ):
            def ptile(pool, shape, dtype, name):
                return pool.tile(shape, dtype, tag=name, name=name)

            # small constants (persistent)
            cos_t = ptile(per, [P, T], F32, "cos")
            nc.sync.dma_start(cos_t, cos2[:])
            sin_t = ptile(per, [P, T], F32, "sin")
            nc.sync.dma_start(sin_t, sin2[:])
            rot_t = ptile(per, [P, P], F32R, "rot")
            nc.sync.dma_start(rot_t, rotm[:])
            onec_t = ptile(per, [P, 1], F32R, "onec")
            nc.sync.dma_start(onec_t, ones_col[:])
            oner_t = ptile(per, [1, P], F32R, "oner")
            nc.sync.dma_start(oner_t, ones_row[:])
            ident_t = ptile(per, [P, P], F32R, "ident")
            nc.sync.dma_start(ident_t, ident[:])

            qn = [ptile(per, [P, T], F32R, f"qn{m}") for m in range(NH)]
            qpeh = [ptile(per, [ROPE, T], F32R, f"qpeh{h}")
                    for h in range(NH)]
            masks = [ptile(per, [P, T], F32R, f"mask{kc}")
                     for kc in range(NKC)]
            aoutT = [ptile(per, [P, T], F32R, f"aout{m}") for m in range(NH)]

            iq_own = ptile(iscg, [P, T], F32, "iqown")
            ik_all = ptile(iscg, [P, S], F32, "ikall")
            isc = [ptile(iscg, [P, S], F32, f"isc{qt}") for qt in range(QT)]
            iscT = [ptile(iscg, [P, T], F32, f"iscT{kc}")
                    for kc in range(NKC)]

            # ================= phase A =================
            with (
                tc.tile_pool(name="ag", bufs=1) as ag,
                tc.tile_pool(name="wpool", bufs=4) as wpool,
                tc.tile_pool(name="tmp", bufs=3) as tmp,
                tc.tile_pool(name="mmps", bufs=3, space="PSUM") as mmps,
                tc.tile_pool(name="smps", bufs=2, space="PSUM") as smps,
            ):
                xts = []
                for k in range(HK):
                    t = ptile(ag, [P, T], F32R, f"xt{k}")
                    nc.sync.dma_start(t, xt[k * P:(k + 1) * P, :])
                    xts.append(t)

                def proj(wdram, m0, m_tiles, out_tiles, rhs_tiles, k_tiles,
                         scale=None, wtag="w"):
                    for m in range(m_tiles):
                        ps = mmps.tile([P, T], F32, tag="mm")
                        for k in range(k_tiles):
                            wt = wpool.tile([P, P], F32R, tag=wtag)
                            nc.sync.dma_start(
                                wt, wdram[k * P:(k + 1) * P,
                                          (m0 + m) * P:(m0 + m + 1) * P])
                            nc.tensor.matmul(ps, wt, rhs_tiles[k],
                                             start=(k == 0),
                                             stop=(k == k_tiles - 1))
                        nc.scalar.activation(
                            out_tiles[m], ps, AF.Copy,
                            scale=(1.0 if scale is None else scale))

                def rmsnorm(tiles, dim):
                    n = len(tiles)
                    ssq = smps.tile([P, T], F32, tag="sm")
                    for i, t in enumerate(tiles):
                        sq = tmp.tile([P, T], F32R, tag="sq")
                        nc.vector.tensor_mul(sq, t, t)
                        nc.tensor.matmul(ssq[:1, :], onec_t, sq,
                                         start=(i == 0), stop=(i == n - 1))
                    sig = tmp.tile([1, T], F32, tag="sig")
                    nc.scalar.activation(sig, ssq[:1, :], AF.Sqrt,
                                         bias=EPS, scale=1.0 / dim)
                    rsig = tmp.tile([1, T], F32R, tag="rsig")
                    nc.vector.reciprocal(rsig, sig)
                    bc_ps = smps.tile([P, T], F32, tag="sm")
                    nc.tensor.matmul(bc_ps, oner_t, rsig,
                                     start=True, stop=True)
                    bc = tmp.tile([P, T], F32, tag="bc")
                    nc.scalar.activation(bc, bc_ps, AF.Copy)
                    for t in tiles:
                        nc.vector.tensor_mul(t, t, bc)

                def rope_apply(dst, src_sb, n_rows):
                    rps = smps.tile([P, T], F32, tag="sm")
                    nc.tensor.matmul(rps[:n_rows, :],
                                     rot_t[:n_rows, :n_rows],
                                     src_sb, start=True, stop=True)
                    a = tmp.tile([P, T], F32, tag="ropea")
                    nc.vector.tensor_mul(a[:n_rows, :], src_sb,
                                         cos_t[:n_rows, :])
                    b = tmp.tile([P, T], F32, tag="ropeb")
                    nc.vector.tensor_mul(b[:n_rows, :], rps[:n_rows, :],
                                         sin_t[:n_rows, :])
                    nc.vector.tensor_add(dst, a[:n_rows, :], b[:n_rows, :])

                # kv_a -> ckv + k_pe
                ckv = [ptile(ag, [P, T], F32R, f"ckv{k}") for k in range(KVK)]
                proj(wkva, 0, KVK, ckv, xts, HK, wtag="wkva")
                kpe_ps = mmps.tile([P, T], F32, tag="mm")
                for k in range(HK):
                    wt = wpool.tile([P, ROPE], F32R, tag="wkpe")
                    nc.sync.dma_start(wt, wkva[k * P:(k + 1) * P,
                                               KVLR:KVLR + ROPE])
                    nc.tensor.matmul(kpe_ps[:ROPE, :], wt, xts[k],
                                     start=(k == 0), stop=(k == HK - 1))
                kpe_sb = tmp.tile([ROPE, T], F32R, tag="kpesb")
                nc.scalar.activation(kpe_sb, kpe_ps[:ROPE, :], AF.Copy)
                rmsnorm(ckv, KVLR)
                kpe_rot = ptile(ag, [ROPE, T], F32R, "kperot")
                rope_apply(kpe_rot, kpe_sb, ROPE)

                # kv_b own tokens -> gather inputs
                for m in range(NH):
                    ps = mmps.tile([P, T], F32, tag="mm")
                    for k in range(KVK):
                        wt = wpool.tile([P, P], F32R, tag="wkbk")
                        nc.sync.dma_start(
                            wt, wkbk[k * P:(k + 1) * P, m * P:(m + 1) * P])
                        nc.tensor.matmul(ps, wt, ckv[k],
                                         start=(k == 0), stop=(k == KVK - 1))
                    kno = tmp.tile([P, T], F32R, tag="kno")
                    nc.scalar.activation(kno, ps, AF.Copy)
                    nc.sync.dma_start(gin1[m * P:(m + 1) * P, :], kno)
                nc.sync.dma_start(gin1[NH * P:NH * P + ROPE, :], kpe_rot)

                for m in range(QT):
                    for n in range(NH * VD // 512):
                        ps = mmps.tile([P, 512], F32, tag="mmv")
                        for k in range(KVK):
                            wt = wpool.tile([P, 512], F32R, tag="wkbv")
                            nc.sync.dma_start(
                                wt, wkbv[k * P:(k + 1) * P,
                                         n * 512:(n + 1) * 512])
                            nc.tensor.matmul(
                                ps, ckv[k][:, m * P:(m + 1) * P], wt,
                                start=(k == 0), stop=(k == KVK - 1))
                        vsb = tmp.tile([P, 512], F32R, tag="vsb")
                        nc.scalar.activation(vsb, ps, AF.Copy)
                        nc.sync.dma_start(
                            gin2[m * P:(m + 1) * P, n * 512:(n + 1) * 512],
                            vsb)

                # indexer projections (true fp32)
                for m, (dst, sc) in enumerate(
                        [(iq_own, ISCALE / INH), (None, 1.0)]):
                    ps = mmps.tile([P, T], F32, tag="mm")
                    for k in range(HK):
                        wt = wpool.tile([P, P], F32, tag="wi")
                        nc.sync.dma_start(
                            wt, wi[k * P:(k + 1) * P, m * P:(m + 1) * P])
                        nc.tensor.matmul(ps, wt, xts[k].bitcast(F32),
                                         start=(k == 0), stop=(k == HK - 1))
                    if dst is None:
                        ik_own = tmp.tile([P, T], F32, tag="ikown")
                        nc.scalar.activation(ik_own, ps, AF.Copy, scale=sc)
                        nc.sync.dma_start(gin1[NH * P + ROPE:G1, :],
                                          ik_own.bitcast(F32R))
                    else:
                        nc.scalar.activation(dst, ps, AF.Copy, scale=sc)

                # all-gather
                if with_collective:
                    nc.gpsimd.collective_compute(
                        "AllGather", OP.bypass,
                        replica_groups=[list(range(NC))],
                        ins=[gin1[:]], outs=[gout1[:]])
                    nc.gpsimd.collective_compute(
                        "AllGather", OP.bypass,
                        replica_groups=[list(range(NC))],
                        ins=[gin2[:]], outs=[gout2[:]])
                else:
                    for u in range(NC):
                        nc.sync.dma_start(gout1[u * G1:(u + 1) * G1, :],
                                          gin1[:])
                        nc.sync.dma_start(gout2[u * T:(u + 1) * T, :],
                                          gin2[:])

                # q path (overlaps the gather)
                qa = [ptile(ag, [P, T], F32R, f"qa{m}") for m in range(QK)]
                proj(wqa, 0, QK, qa, xts, HK, wtag="wqa")
                rmsnorm(qa, QLR)
                proj(wqb, 0, NH, qn, qa, QK, scale=SCALE, wtag="wqb")
                for m in range(NH // 2):
                    ps = mmps.tile([P, T], F32, tag="mm")
                    for k in range(QK):
                        wt = wpool.tile([P, P], F32R, tag="wqb")
                        nc.sync.dma_start(
                            wt, wqb[k * P:(k + 1) * P,
                                    (NH + m) * P:(NH + m + 1) * P])
                        nc.tensor.matmul(ps, wt, qa[k],
                                         start=(k == 0), stop=(k == QK - 1))
                    praw = tmp.tile([P, T], F32R, tag="qperaw")
                    nc.scalar.activation(praw, ps, AF.Copy, scale=SCALE)
                    prot = tmp.tile([P, T], F32R, tag="qperot")
                    rope_apply(prot, praw, P)
                    # split the two 64-row head halves to partition base 0
                    nc.sync.dma_start(qpeh[2 * m], prot[:ROPE, :])
                    nc.sync.dma_start(qpeh[2 * m + 1], prot[ROPE:, :])

                # iscores in both layouts from gathered ik
                for kc in range(NKC):
                    u, half = kc % NC, kc // NC
                    nc.sync.dma_start(
                        ik_all[:, kc * P:(kc + 1) * P],
                        gout1.bitcast(F32)[u * G1 + NH * P + ROPE:
                                           u * G1 + G1,
                                           half * P:(half + 1) * P])
                for qt in range(QT):
                    for n in range(S // 512):
                        ps = mmps.tile([P, 512], F32, tag="mmv")
                        nc.tensor.matmul(
                            ps, iq_own[:, qt * P:(qt + 1) * P],
                            ik_all[:, n * 512:(n + 1) * 512],
                            start=True, stop=True)
                        nc.scalar.activation(
                            isc[qt][:, n * 512:(n + 1) * 512], ps, AF.Copy)
                for kc in range(NKC):
                    ps = mmps.tile([P, T], F32, tag="mm")
                    nc.tensor.matmul(ps, ik_all[:, kc * P:(kc + 1) * P],
                                     iq_own, start=True, stop=True)
                    nc.scalar.activation(iscT[kc], ps, AF.Copy)

            # ================= phase B: bisection + masks =================
            with (
                tc.tile_pool(name="bis", bufs=1) as bis,
                tc.tile_pool(name="bps", bufs=2, space="PSUM") as bps,
            ):
                scratch = bis.tile([P, S], F32, tag="scratch")
                lo = bis.tile([P, QT], F32, tag="lo")
                hi = bis.tile([P, QT], F32, tag="hi")
                mid = bis.tile([P, QT], F32, tag="mid")
                cnt = bis.tile([P, QT], F32, tag="cnt")
                pred = bis.tile([P, QT], mybir.dt.uint32, tag="pred")
                npred = bis.tile([P, QT], mybir.dt.uint32, tag="npred")
                s1 = bis.tile([P, QT], F32, tag="s1")
                s2 = bis.tile([P, QT], F32, tag="s2")
                sgm = bis.tile([P, QT], F32, tag="sgm")
                for qt in range(QT):
                    nc.vector.tensor_reduce(
                        s1[:, qt:qt + 1], isc[qt], mybir.AxisListType.X,
                        op=OP.add)
                    nc.vector.tensor_mul(scratch, isc[qt], isc[qt])
                    nc.vector.tensor_reduce(
                        s2[:, qt:qt + 1], scratch, mybir.AxisListType.X,
                        op=OP.add)
                nc.vector.tensor_scalar_mul(s1, s1, 1.0 / S)
                nc.vector.tensor_scalar_mul(s2, s2, 1.0 / S)
                nc.vector.tensor_mul(mid, s1, s1)
                nc.vector.tensor_sub(s2, s2, mid)
                nc.scalar.activation(sgm, s2, AF.Sqrt, bias=1e-12)
                nc.vector.tensor_scalar(lo, sgm, 0.25, None, op0=OP.mult)
                nc.vector.tensor_add(lo, lo, s1)
                nc.vector.tensor_scalar(hi, sgm, 1.15, None, op0=OP.mult)
                nc.vector.tensor_add(hi, hi, s1)
                for _ in range(BISECT_ITERS):
                    nc.vector.tensor_add(mid, lo, hi)
                    nc.vector.tensor_scalar_mul(mid, mid, 0.5)
                    for qt in range(QT):
                        nc.vector.tensor_scalar(
                            scratch, isc[qt], mid[:, qt:qt + 1], None,
                            op0=OP.is_ge, op1=OP.add,
                            accum_out=cnt[:, qt:qt + 1])
                    nc.vector.tensor_scalar(pred, cnt, float(ITOPK), None,
                                            op0=OP.is_ge)
                    nc.vector.tensor_scalar(npred, cnt, float(ITOPK), None,
                                            op0=OP.is_lt)
                    nc.vector.copy_predicated(lo, pred, mid)
                    nc.vector.copy_predicated(hi, npred, mid)
                # threshold -> row layout [1, T] (fp32 matmuls, tiny)
                thr_row = bis.tile([1, T], F32, tag="throw")
                for qt in range(QT):
                    tp = bps.tile([1, P], F32, tag="thr")
                    nc.tensor.matmul(tp, lo[:, qt:qt + 1],
                                     ident_t.bitcast(F32),
                                     start=True, stop=True)
                    nc.scalar.activation(thr_row[:, qt * P:(qt + 1) * P],
                                         tp, AF.Copy)
                tbc_ps = bps.tile([P, T], F32, tag="thrbc")
                nc.tensor.matmul(tbc_ps, oner_t.bitcast(F32), thr_row,
                                 start=True, stop=True)
                thr_bc = bis.tile([P, T], F32, tag="thrbcsb")
                nc.scalar.activation(thr_bc, tbc_ps, AF.Copy)
                for kc in range(NKC):
                    ct = bis.tile([P, T], F32R, tag="causal")
                    nc.sync.dma_start(ct, causal[kc * P:(kc + 1) * P, :])
                    nc.vector.tensor_tensor(
                        out=masks[kc], in0=iscT[kc], in1=thr_bc, op=OP.is_ge)
                    nc.vector.tensor_mul(masks[kc], masks[kc], ct)

        # ================= phase C: attention =================
        # (iscg closed: isc/iscT freed)
            with (
                tc.tile_pool(name="kvp", bufs=2) as kvp,
                tc.tile_pool(name="ap", bufs=3) as ap,
                tc.tile_pool(name="scps", bufs=2, space="PSUM") as scps,
                tc.tile_pool(name="dps", bufs=2, space="PSUM") as dps,
                tc.tile_pool(name="ops", bufs=2, space="PSUM") as ops,
                tc.tile_pool(name="mps", bufs=2, space="PSUM") as mps,
            ):
                for h in range(NH):
                    kn = kvp.tile([P, S], F32R, tag="kn")
                    vv = kvp.tile([P, NKC, VD], F32R, tag="vv")
                    kpe_g = kvp.tile([ROPE, S], F32R, tag="kpeg")
                    for kc in range(NKC):
                        u, half = kc % NC, kc // NC
                        nc.sync.dma_start(
                            kn[:, kc * P:(kc + 1) * P],
                            gout1[u * G1 + h * P:u * G1 + (h + 1) * P,
                                  half * P:(half + 1) * P])
                        nc.sync.dma_start(
                            vv[:, kc, :],
                            gout2[u * T + half * P:u * T + (half + 1) * P,
                                  h * VD:(h + 1) * VD])
                        nc.sync.dma_start(
                            kpe_g[:, kc * P:(kc + 1) * P],
                            gout1[u * G1 + NH * P:u * G1 + NH * P + ROPE,
                                  half * P:(half + 1) * P])
                    # column-sum of V over all keys (empty-mask correction)
                    csr_ps = mps.tile([P, P], F32, tag="misc")
                    for kc in range(NKC):
                        nc.tensor.matmul(csr_ps[:1, :], onec_t,
                                         vv[:, kc, :],
                                         start=(kc == 0),
                                         stop=(kc == NKC - 1))
                    csr = ap.tile([1, P], F32R, tag="csr")
                    nc.scalar.activation(csr, csr_ps[:1, :], AF.Copy)

                    for qt in range(QT):
                        nkc = NKC if qt == 1 else NKC // 2
                        dsum = dps.tile([1, P], F32, tag="dsum")
                        opsum = ops.tile([P, P], F32, tag="opsum")
                        for kc in range(nkc):
                            sc = scps.tile([P, P], F32, tag="sc")
                            nc.tensor.matmul(
                                sc, kn[:, kc * P:(kc + 1) * P],
                                qn[h][:, qt * P:(qt + 1) * P],
                                start=True, stop=False)
                            nc.tensor.matmul(
                                sc, kpe_g[:, kc * P:(kc + 1) * P],
                                qpeh[h][:, qt * P:(qt + 1) * P],
                                start=False, stop=True)
                            at = ap.tile([P, P], F32R, tag="at")
                            nc.scalar.activation(at, sc, AF.Exp)
                            nc.vector.tensor_mul(
                                at, at, masks[kc][:, qt * P:(qt + 1) * P])
                            nc.tensor.matmul(dsum, onec_t, at,
                                             start=(kc == 0),
                                             stop=(kc == nkc - 1))
                            nc.tensor.matmul(opsum, vv[:, kc, :], at,
                                             start=(kc == 0),
                                             stop=(kc == nkc - 1))
                        u01 = ap.tile([1, P], F32, tag="u01")
                        nc.vector.tensor_scalar(u01, dsum, 0.0, None,
                                                op0=OP.is_equal)
                        den = ap.tile([1, P], F32, tag="den")
                        nc.vector.tensor_add(den, dsum, u01)
                        rec = ap.tile([1, P], F32R, tag="rec")
                        nc.vector.reciprocal(rec, den)
                        u01s = ap.tile([1, P], F32R, tag="u01s")
                        nc.vector.tensor_scalar_mul(u01s, u01, 1.0 / S)
                        nc.tensor.matmul(opsum, csr, u01s,
                                         start=False, stop=True,
                                         skip_group_check=True)
                        bc_ps = mps.tile([P, P], F32, tag="misc")
                        nc.tensor.matmul(bc_ps, oner_t, rec,
                                         start=True, stop=True)
                        bc = ap.tile([P, P], F32, tag="recbc")
                        nc.scalar.activation(bc, bc_ps, AF.Copy)
                        nc.vector.tensor_mul(
                            aoutT[h][:, qt * P:(qt + 1) * P], opsum, bc)

            # ================= phase D: o_proj =================
            with (
                tc.tile_pool(name="wp2", bufs=4) as wp2,
                tc.tile_pool(name="op2", bufs=3) as op2,
                tc.tile_pool(name="mmps2", bufs=3, space="PSUM") as mmps2,
            ):
                for m in range(HK):
                    ps = mmps2.tile([P, T], F32, tag="mm2")
                    for k in range(NH):
                        wt = wp2.tile([P, P], F32R, tag="wo")
                        nc.sync.dma_start(
                            wt, wo[k * P:(k + 1) * P, m * P:(m + 1) * P])
                        nc.tensor.matmul(ps, wt, aoutT[k],
                                         start=(k == 0), stop=(k == NH - 1))
                    osb = op2.tile([P, T], F32, tag="osb")
                    nc.scalar.activation(osb, ps, AF.Copy)
                    nc.sync.dma_start(outT[m * P:(m + 1) * P, :], osb)

    split_multi_waits(nc)
    return nc


_BUILT = {}


def kernel(**inputs):
    per_core = host_prep(inputs)
    if "nc" not in _BUILT:
        _BUILT["nc"] = build_kernel(with_collective=True)
    nc = _BUILT["nc"]
    res = run_bass_kernel_spmd(nc, per_core, core_ids=list(range(NC)))
    results = [{"outT": r["outT"]} for r in res.results]
    return unshard_output(results)


# revision 11
# speedup vs baseline: 1.0235x; 1.0235x over previous
"""MiniLLM MLA sparse attention on 8 Trainium2 NeuronCores.

Sharding: cyclic over tokens (core c owns tokens t with t % 8 == c), which
makes the causal attention work identical on every core (SPMD-uniform: one
NEFF for all cores). Activations live in [feature, token] layout so matmuls
chain without transposes; all weights are pre-transposed / reordered /
norm-folded on the host. Keys are processed in "gathered" (core-major,
permuted) order; softmax, top-k counting and attn@V are permutation-invariant
over keys, and causality is applied via a host-precomputed mask in the
permuted order.

Numerics: float32r (TF32-like, full PE speed) for all matmuls except the
indexer path, which stays true fp32 so the top-512 selection matches the
reference; the per-row top-k threshold is found by bisection with a fused
compare+count DVE op; softmax skips max-subtraction (scores are O(1)); rows
whose allowed set is empty reproduce the reference's uniform-softmax via a
mean-of-V rank-1 correction.
"""
import numpy as np

import concourse.bass as bass
import concourse.mybir as mybir
import concourse.tile as tile
from concourse.bass_utils import run_bass_kernel_spmd
from wait_split import split_multi_waits

# Problem dims
B, S, H = 1, 2048, 2048
NH = 16
NOPE, ROPE, VD = 128, 64, 128
QD = NOPE + ROPE
QLR, KVLR = 1536, 512
INH, IND, ITOPK = 4, 32, 512
EPS = 1e-6
SCALE = QD ** -0.5
ISCALE = IND ** -0.5

NC = 8            # cores
T = S // NC       # tokens per core (256)
P = 128
NKC = S // P      # key chunks (16)
QT = T // P       # query tiles per core (2)
BISECT_ITERS = 26

HK = H // P       # 16
QK = QLR // P     # 12
KVK = KVLR // P   # 4
G1 = NH * NOPE + ROPE + P   # gather-1 rows: knope + kpe + ik = 2240

F32 = mybir.dt.float32
F32R = mybir.dt.float32r
AF = mybir.ActivationFunctionType
OP = mybir.AluOpType


def _dt(t):
    return np.ascontiguousarray(t, dtype=np.float32)


def host_prep(inputs):
    """Split/reorder/transpose on the host. Returns per-core input maps."""
    x = np.asarray(inputs["x"])[0]                     # [S, H]
    cos = np.asarray(inputs["cos"])                    # [S, ROPE]
    sin = np.asarray(inputs["sin"])
    q_a_w = np.asarray(inputs["q_a_w"])                # [QLR, H]
    q_a_norm_w = np.asarray(inputs["q_a_norm_w"])      # [QLR]
    q_b_w = np.asarray(inputs["q_b_w"])                # [NH*QD, QLR]
    kv_a_w = np.asarray(inputs["kv_a_w"])              # [KVLR+ROPE, H]
    kv_a_norm_w = np.asarray(inputs["kv_a_norm_w"])    # [KVLR]
    kv_b_w = np.asarray(inputs["kv_b_w"])              # [NH*(NOPE+VD), KVLR]
    o_w = np.asarray(inputs["o_w"])                    # [H, NH*VD]
    idx_q_w = np.asarray(inputs["idx_q_w"])            # [INH*IND, H]
    idx_k_w = np.asarray(inputs["idx_k_w"])

    q_b_f = q_b_w * q_a_norm_w[None, :]
    kv_b_f = kv_b_w * kv_a_norm_w[None, :]

    qb3 = q_b_f.reshape(NH, QD, QLR)
    qb_nope = qb3[:, :NOPE, :].reshape(NH * NOPE, QLR)
    qb_rope = qb3[:, NOPE:, :].reshape(NH * ROPE, QLR)
    wqb = _dt(np.concatenate([qb_nope, qb_rope], 0).T)       # [QLR, 3072]

    kb3 = kv_b_f.reshape(NH, NOPE + VD, KVLR)
    wkbk = _dt(kb3[:, :NOPE, :].reshape(NH * NOPE, KVLR).T)  # [KVLR, 2048]
    wkbv = _dt(kb3[:, NOPE:, :].reshape(NH * VD, KVLR).T)    # [KVLR, 2048]

    wqa = _dt(q_a_w.T)                                  # [H, QLR]
    wkva = _dt(kv_a_w.T)                                # [H, KVLR+ROPE]
    wo = _dt(o_w.T)                                     # [NH*VD, H]
    wi = _dt(np.concatenate([idx_q_w, idx_k_w], 0).T)   # [H, 256]

    # rotate_half as matmul: rot = A @ v per 64-block; lhsT = A.T
    A64 = np.zeros((ROPE, ROPE), np.float32)
    A64[:32, 32:] = -np.eye(32, dtype=np.float32)
    A64[32:, :32] = np.eye(32, dtype=np.float32)
    A128 = np.zeros((P, P), np.float32)
    A128[:64, :64] = A64
    A128[64:, 64:] = A64
    rotm = _dt(A128.T)

    ones_col = np.ones((P, 1), np.float32)
    ones_row = np.ones((1, P), np.float32)
    ident = np.eye(P, dtype=np.float32)

    # permuted-order global key index: chunk kc -> core u = kc % 8,
    # half = kc // 8; row i -> global token u + 8*(half*128 + i)
    gkey = np.empty((S,), np.int64)
    for kc in range(NKC):
        u, half = kc % NC, kc // NC
        gkey[kc * P:(kc + 1) * P] = u + NC * (half * P + np.arange(P))

    xT = x.T
    per_core = []
    for c in range(NC):
        toks = np.arange(c, S, NC)
        cosc = cos[toks].T
        sinc = sin[toks].T
        gq = c + NC * np.arange(T)
        causalT = (gkey[:, None] <= gq[None, :]).astype(np.float32)  # [S, T]
        per_core.append({
            "xt": _dt(xT[:, toks]),
            "cos2": _dt(np.concatenate([cosc, cosc], 0)),
            "sin2": _dt(np.concatenate([sinc, sinc], 0)),
            "causal": _dt(causalT),
            "wqa": wqa, "wqb": wqb, "wkva": wkva,
            "wkbk": wkbk, "wkbv": wkbv, "wo": wo, "wi": wi,
            "rotm": rotm, "ones_col": ones_col, "ones_row": ones_row,
            "ident": ident,
        })
    return per_core


def unshard_output(results):
    out = np.empty((B, S, H), np.float32)
    for c in range(NC):
        toks = np.arange(c, S, NC)
        out[0, toks, :] = results[c]["outT"].T
    return out


def build_kernel(with_collective=True):
    nc = bass.Bass(trn_type="TRN2", num_devices=NC)

    for _cv in (EPS, 1e-12):
        _ct = nc.alloc_sbuf_tensor(f"const-f32-{_cv}", [128, 1], F32)
        nc.gpsimd.memset(_ct.ap(), _cv)
        nc.const_aps.aps[(F32, _cv)] = _ct.ap()
    nc.all_engine_barrier()

    def din(name, shape, dtype=F32R):
        return nc.dram_tensor(name, shape, dtype, kind="ExternalInput")

    xt = din("xt", [H, T])
    cos2 = din("cos2", [P, T], F32)
    sin2 = din("sin2", [P, T], F32)
    causal = din("causal", [S, T])
    wqa = din("wqa", [H, QLR])
    wqb = din("wqb", [QLR, NH * QD])
    wkva = din("wkva", [H, KVLR + ROPE])
    wkbk = din("wkbk", [KVLR, NH * NOPE])
    wkbv = din("wkbv", [KVLR, NH * VD])
    wo = din("wo", [NH * VD, H])
    wi = din("wi", [H, 2 * INH * IND], F32)
    rotm = din("rotm", [P, P])
    ones_col = din("ones_col", [P, 1])
    ones_row = din("ones_row", [1, P])
    ident = din("ident", [P, P])

    outT = nc.dram_tensor("outT", [H, T], F32, kind="ExternalOutput")

    # internal DRAM gather buffers
    gin1 = nc.dram_tensor("gin1", [G1, T], F32R)
    gout1 = nc.dram_tensor("gout1", [NC * G1, T], F32R, addr_space="Shared")
    gin2 = nc.dram_tensor("gin2", [T, NH * VD], F32R)
    gout2 = nc.dram_tensor("gout2", [NC * T, NH * VD], F32R,
                           addr_space="Shared")

    with tile.TileContext(nc) as tc:
        with (
            nc.allow_low_precision(reason="float32r is 4-byte; intentional"),
            tc.tile_pool(name="per", bufs=1) as per,
            tc.tile_pool(name="iscg", bufs=1) as iscg,
        ):
            def ptile(pool, shape, dtype, name):
                return pool.tile(shape, dtype, tag=name, name=name)

            # small constants (persistent)
            cos_t = ptile(per, [P, T], F32, "cos")
            nc.sync.dma_start(cos_t, cos2[:])
            sin_t = ptile(per, [P, T], F32, "sin")
            nc.sync.dma_start(sin_t, sin2[:])
            rot_t = ptile(per, [P, P], F32R, "rot")
            nc.sync.dma_start(rot_t, rotm[:])
            onec_t = ptile(per, [P, 1], F32R, "onec")
            nc.sync.dma_start(onec_t, ones_col[:])
            oner_t = ptile(per, [1, P], F32R, "oner")
            nc.sync.dma_start(oner_t, ones_row[:])
            ident_t = ptile(per, [P, P], F32R, "ident")
            nc.sync.dma_start(ident_t, ident[:])

            qn = [ptile(per, [P, T], F32R, f"qn{m}") for m in range(NH)]
            qpeh = [ptile(per, [ROPE, T], F32R, f"qpeh{h}")
                    for h in range(NH)]
            masks = [ptile(per, [P, T], F32R, f"mask{kc}")
                     for kc in range(NKC)]
            aoutT = [ptile(per, [P, T], F32R, f"aout{m}") for m in range(NH)]

            iq_own = ptile(iscg, [P, T], F32, "iqown")
            ik_all = ptile(iscg, [P, S], F32, "ikall")
            isc = [ptile(iscg, [P, S], F32, f"isc{qt}") for qt in range(QT)]

            # ================= phase A =================
            with (
                tc.tile_pool(name="ag", bufs=1) as ag,
                tc.tile_pool(name="wpool", bufs=4) as wpool,
                tc.tile_pool(name="tmp", bufs=3) as tmp,
                tc.tile_pool(name="mmps", bufs=3, space="PSUM") as mmps,
                tc.tile_pool(name="smps", bufs=2, space="PSUM") as smps,
            ):
                xts = []
                for k in range(HK):
                    t = ptile(ag, [P, T], F32R, f"xt{k}")
                    nc.sync.dma_start(t, xt[k * P:(k + 1) * P, :])
                    xts.append(t)

                def proj(wdram, m0, m_tiles, out_tiles, rhs_tiles, k_tiles,
                         scale=None, wtag="w"):
                    for m in range(m_tiles):
                        ps = mmps.tile([P, T], F32, tag="mm")
                        for k in range(k_tiles):
                            wt = wpool.tile([P, P], F32R, tag=wtag)
                            nc.sync.dma_start(
                                wt, wdram[k * P:(k + 1) * P,
                                          (m0 + m) * P:(m0 + m + 1) * P])
                            nc.tensor.matmul(ps, wt, rhs_tiles[k],
                                             start=(k == 0),
                                             stop=(k == k_tiles - 1))
                        nc.scalar.activation(
                            out_tiles[m], ps, AF.Copy,
                            scale=(1.0 if scale is None else scale))

                def rmsnorm(tiles, dim):
                    n = len(tiles)
                    ssq = smps.tile([P, T], F32, tag="sm")
                    for i, t in enumerate(tiles):
                        sq = tmp.tile([P, T], F32R, tag="sq")
                        nc.vector.tensor_mul(sq, t, t)
                        nc.tensor.matmul(ssq[:1, :], onec_t, sq,
                                         start=(i == 0), stop=(i == n - 1))
                    sig = tmp.tile([1, T], F32, tag="sig")
                    nc.scalar.activation(sig, ssq[:1, :], AF.Sqrt,
                                         bias=EPS, scale=1.0 / dim)
                    rsig = tmp.tile([1, T], F32R, tag="rsig")
                    nc.vector.reciprocal(rsig, sig)
                    bc_ps = smps.tile([P, T], F32, tag="sm")
                    nc.tensor.matmul(bc_ps, oner_t, rsig,
                                     start=True, stop=True)
                    bc = tmp.tile([P, T], F32, tag="bc")
                    nc.scalar.activation(bc, bc_ps, AF.Copy)
                    for t in tiles:
                        nc.vector.tensor_mul(t, t, bc)

                def rope_apply(dst, src_sb, n_rows):
                    rps = smps.tile([P, T], F32, tag="sm")
                    nc.tensor.matmul(rps[:n_rows, :],
                                     rot_t[:n_rows, :n_rows],
                                     src_sb, start=True, stop=True)
                    a = tmp.tile([P, T], F32, tag="ropea")
                    nc.vector.tensor_mul(a[:n_rows, :], src_sb,
                                         cos_t[:n_rows, :])
                    b = tmp.tile([P, T], F32, tag="ropeb")
                    nc.vector.tensor_mul(b[:n_rows, :], rps[:n_rows, :],
                                         sin_t[:n_rows, :])
                    nc.vector.tensor_add(dst, a[:n_rows, :], b[:n_rows, :])

                # kv_a -> ckv + k_pe
                ckv = [ptile(ag, [P, T], F32R, f"ckv{k}") for k in range(KVK)]
                proj(wkva, 0, KVK, ckv, xts, HK, wtag="wkva")
                kpe_ps = mmps.tile([P, T], F32, tag="mm")
                for k in range(HK):
                    wt = wpool.tile([P, ROPE], F32R, tag="wkpe")
                    nc.sync.dma_start(wt, wkva[k * P:(k + 1) * P,
                                               KVLR:KVLR + ROPE])
                    nc.tensor.matmul(kpe_ps[:ROPE, :], wt, xts[k],
                                     start=(k == 0), stop=(k == HK - 1))
                kpe_sb = tmp.tile([ROPE, T], F32R, tag="kpesb")
                nc.scalar.activation(kpe_sb, kpe_ps[:ROPE, :], AF.Copy)
                rmsnorm(ckv, KVLR)
                kpe_rot = ptile(ag, [ROPE, T], F32R, "kperot")
                rope_apply(kpe_rot, kpe_sb, ROPE)

                # kv_b own tokens -> gather inputs
                for m in range(NH):
                    ps = mmps.tile([P, T], F32, tag="mm")
                    for k in range(KVK):
                        wt = wpool.tile([P, P], F32R, tag="wkbk")
                        nc.sync.dma_start(
                            wt, wkbk[k * P:(k + 1) * P, m * P:(m + 1) * P])
                        nc.tensor.matmul(ps, wt, ckv[k],
                                         start=(k == 0), stop=(k == KVK - 1))
                    kno = tmp.tile([P, T], F32R, tag="kno")
                    nc.scalar.activation(kno, ps, AF.Copy)
                    nc.sync.dma_start(gin1[m * P:(m + 1) * P, :], kno)
                nc.sync.dma_start(gin1[NH * P:NH * P + ROPE, :], kpe_rot)

                for m in range(QT):
                    for n in range(NH * VD // 512):
                        ps = mmps.tile([P, 512], F32, tag="mmv")
                        for k in range(KVK):
                            wt = wpool.tile([P, 512], F32R, tag="wkbv")
                            nc.sync.dma_start(
                                wt, wkbv[k * P:(k + 1) * P,
                                         n * 512:(n + 1) * 512])
                            nc.tensor.matmul(
                                ps, ckv[k][:, m * P:(m + 1) * P], wt,
                                start=(k == 0), stop=(k == KVK - 1))
                        vsb = tmp.tile([P, 512], F32R, tag="vsb")
                        nc.scalar.activation(vsb, ps, AF.Copy)
                        nc.sync.dma_start(
                            gin2[m * P:(m + 1) * P, n * 512:(n + 1) * 512],
                            vsb)

                # indexer projections (true fp32)
                for m, (dst, sc) in enumerate(
                        [(iq_own, ISCALE / INH), (None, 1.0)]):
                    ps = mmps.tile([P, T], F32, tag="mm")
                    for k in range(HK):
                        wt = wpool.tile([P, P], F32, tag="wi")
                        nc.sync.dma_start(
                            wt, wi[k * P:(k + 1) * P, m * P:(m + 1) * P])
                        nc.tensor.matmul(ps, wt, xts[k].bitcast(F32),
                                         start=(k == 0), stop=(k == HK - 1))
                    if dst is None:
                        ik_own = tmp.tile([P, T], F32, tag="ikown")
                        nc.scalar.activation(ik_own, ps, AF.Copy, scale=sc)
                        nc.sync.dma_start(gin1[NH * P + ROPE:G1, :],
                                          ik_own.bitcast(F32R))
                    else:
                        nc.scalar.activation(dst, ps, AF.Copy, scale=sc)

                # all-gather
                if with_collective:
                    nc.gpsimd.collective_compute(
                        "AllGather", OP.bypass,
                        replica_groups=[list(range(NC))],
                        ins=[gin1[:]], outs=[gout1[:]])
                    nc.gpsimd.collective_compute(
                        "AllGather", OP.bypass,
                        replica_groups=[list(range(NC))],
                        ins=[gin2[:]], outs=[gout2[:]])
                else:
                    for u in range(NC):
                        nc.sync.dma_start(gout1[u * G1:(u + 1) * G1, :],
                                          gin1[:])
                        nc.sync.dma_start(gout2[u * T:(u + 1) * T, :],
                                          gin2[:])

                # q path (overlaps the gather)
                qa = [ptile(ag, [P, T], F32R, f"qa{m}") for m in range(QK)]
                proj(wqa, 0, QK, qa, xts, HK, wtag="wqa")
                rmsnorm(qa, QLR)
                proj(wqb, 0, NH, qn, qa, QK, scale=SCALE, wtag="wqb")
                for m in range(NH // 2):
                    ps = mmps.tile([P, T], F32, tag="mm")
                    for k in range(QK):
                        wt = wpool.tile([P, P], F32R, tag="wqb")
                        nc.sync.dma_start(
                            wt, wqb[k * P:(k + 1) * P,
                                    (NH + m) * P:(NH + m + 1) * P])
                        nc.tensor.matmul(ps, wt, qa[k],
                                         start=(k == 0), stop=(k == QK - 1))
                    praw = tmp.tile([P, T], F32R, tag="qperaw")
                    nc.scalar.activation(praw, ps, AF.Copy, scale=SCALE)
                    prot = tmp.tile([P, T], F32R, tag="qperot")
                    rope_apply(prot, praw, P)
                    # split the two 64-row head halves to partition base 0
                    nc.sync.dma_start(qpeh[2 * m], prot[:ROPE, :])
                    nc.sync.dma_start(qpeh[2 * m + 1], prot[ROPE:, :])

                # iscores in both layouts from gathered ik
                for kc in range(NKC):
                    u, half = kc % NC, kc // NC
                    nc.sync.dma_start(
                        ik_all[:, kc * P:(kc + 1) * P],
                        gout1.bitcast(F32)[u * G1 + NH * P + ROPE:
                                           u * G1 + G1,
                                           half * P:(half + 1) * P])
                for qt in range(QT):
                    for n in range(S // 512):
                        ps = mmps.tile([P, 512], F32, tag="mmv")
                        nc.tensor.matmul(
                            ps, iq_own[:, qt * P:(qt + 1) * P],
                            ik_all[:, n * 512:(n + 1) * 512],
                            start=True, stop=True)
                        nc.scalar.activation(
                            isc[qt][:, n * 512:(n + 1) * 512], ps, AF.Copy)

            # ================= phase B: bisection + masks =================
            with (
                tc.tile_pool(name="bis", bufs=1) as bis,
                tc.tile_pool(name="bps", bufs=2, space="PSUM") as bps,
            ):
                scratch = bis.tile([P, S], F32, tag="scratch")
                lo = bis.tile([P, QT], F32, tag="lo")
                hi = bis.tile([P, QT], F32, tag="hi")
                mid = bis.tile([P, QT], F32, tag="mid")
                cnt = bis.tile([P, QT], F32, tag="cnt")
                pred = bis.tile([P, QT], mybir.dt.uint32, tag="pred")
                npred = bis.tile([P, QT], mybir.dt.uint32, tag="npred")
                s1 = bis.tile([P, QT], F32, tag="s1")
                s2 = bis.tile([P, QT], F32, tag="s2")
                sgm = bis.tile([P, QT], F32, tag="sgm")
                for qt in range(QT):
                    nc.vector.tensor_reduce(
                        s1[:, qt:qt + 1], isc[qt], mybir.AxisListType.X,
                        op=OP.add)
                    nc.vector.tensor_mul(scratch, isc[qt], isc[qt])
                    nc.vector.tensor_reduce(
                        s2[:, qt:qt + 1], scratch, mybir.AxisListType.X,
                        op=OP.add)
                nc.vector.tensor_scalar_mul(s1, s1, 1.0 / S)
                nc.vector.tensor_scalar_mul(s2, s2, 1.0 / S)
                nc.vector.tensor_mul(mid, s1, s1)
                nc.vector.tensor_sub(s2, s2, mid)
                nc.scalar.activation(sgm, s2, AF.Sqrt, bias=1e-12)
                nc.vector.tensor_scalar(lo, sgm, 0.25, None, op0=OP.mult)
                nc.vector.tensor_add(lo, lo, s1)
                nc.vector.tensor_scalar(hi, sgm, 1.15, None, op0=OP.mult)
                nc.vector.tensor_add(hi, hi, s1)
                for _ in range(BISECT_ITERS):
                    nc.vector.tensor_add(mid, lo, hi)
                    nc.vector.tensor_scalar_mul(mid, mid, 0.5)
                    for qt in range(QT):
                        nc.vector.tensor_scalar(
                            scratch, isc[qt], mid[:, qt:qt + 1], None,
                            op0=OP.is_ge, op1=OP.add,
                            accum_out=cnt[:, qt:qt + 1])
                    nc.vector.tensor_scalar(pred, cnt, float(ITOPK), None,
                                            op0=OP.is_ge)
                    nc.vector.tensor_scalar(npred, cnt, float(ITOPK), None,
                                            op0=OP.is_lt)
                    nc.vector.copy_predicated(lo, pred, mid)
                    nc.vector.copy_predicated(hi, npred, mid)
                # mask from the SAME isc tensor the counts used (exact
                # 512-selection), then PE-transpose the 0/1 mask to
                # [keys, q] layout (transpose of 0/1 values is exact)
                cts = []
                for kc in range(NKC):
                    ct = bis.tile([P, T], F32R, tag=f"causal{kc}",
                                  name=f"causal{kc}")
                    nc.sync.dma_start(ct, causal[kc * P:(kc + 1) * P, :])
                    cts.append(ct)
                for qt in range(QT):
                    m01 = bis.tile([P, S], F32, tag="m01", name="m01")
                    nc.vector.tensor_scalar(m01, isc[qt], lo[:, qt:qt + 1],
                                            None, op0=OP.is_ge)
                    for kc in range(NKC):
                        tp = bps.tile([P, P], F32, tag="tp", name="tp")
                        nc.tensor.transpose(
                            tp, m01[:, kc * P:(kc + 1) * P],
                            ident_t.bitcast(F32))
                        nc.vector.tensor_mul(
                            masks[kc][:, qt * P:(qt + 1) * P], tp,
                            cts[kc][:, qt * P:(qt + 1) * P])

        # ================= phase C: attention =================
        # (iscg closed: isc/iscT freed)
            with (
                tc.tile_pool(name="kvp", bufs=2) as kvp,
                tc.tile_pool(name="ap", bufs=3) as ap,
                tc.tile_pool(name="scps", bufs=2, space="PSUM") as scps,
                tc.tile_pool(name="dps", bufs=2, space="PSUM") as dps,
                tc.tile_pool(name="ops", bufs=2, space="PSUM") as ops,
                tc.tile_pool(name="mps", bufs=2, space="PSUM") as mps,
            ):
                for h in range(NH):
                    kn = kvp.tile([P, S], F32R, tag="kn")
                    vv = kvp.tile([P, NKC, VD], F32R, tag="vv")
                    kpe_g = kvp.tile([ROPE, S], F32R, tag="kpeg")
                    for kc in range(NKC):
                        u, half = kc % NC, kc // NC
                        nc.sync.dma_start(
                            kn[:, kc * P:(kc + 1) * P],
                            gout1[u * G1 + h * P:u * G1 + (h + 1) * P,
                                  half * P:(half + 1) * P])
                        nc.sync.dma_start(
                            vv[:, kc, :],
                            gout2[u * T + half * P:u * T + (half + 1) * P,
                                  h * VD:(h + 1) * VD])
                        nc.sync.dma_start(
                            kpe_g[:, kc * P:(kc + 1) * P],
                            gout1[u * G1 + NH * P:u * G1 + NH * P + ROPE,
                                  half * P:(half + 1) * P])
                    # column-sum of V over all keys (empty-mask correction)
                    csr_ps = mps.tile([P, P], F32, tag="misc")
                    for kc in range(NKC):
                        nc.tensor.matmul(csr_ps[:1, :], onec_t,
                                         vv[:, kc, :],
                                         start=(kc == 0),
                                         stop=(kc == NKC - 1))
                    csr = ap.tile([1, P], F32R, tag="csr")
                    nc.scalar.activation(csr, csr_ps[:1, :], AF.Copy)

                    for qt in range(QT):
                        nkc = NKC if qt == 1 else NKC // 2
                        dsum = dps.tile([1, P], F32, tag="dsum")
                        opsum = ops.tile([P, P], F32, tag="opsum")
                        for kc in range(nkc):
                            sc = scps.tile([P, P], F32, tag="sc")
                            nc.tensor.matmul(
                                sc, kn[:, kc * P:(kc + 1) * P],
                                qn[h][:, qt * P:(qt + 1) * P],
                                start=True, stop=False)
                            nc.tensor.matmul(
                                sc, kpe_g[:, kc * P:(kc + 1) * P],
                                qpeh[h][:, qt * P:(qt + 1) * P],
                                start=False, stop=True)
                            at = ap.tile([P, P], F32R, tag="at")
                            nc.scalar.activation(at, sc, AF.Exp)
                            nc.vector.tensor_mul(
                                at, at, masks[kc][:, qt * P:(qt + 1) * P])
                            nc.tensor.matmul(dsum, onec_t, at,
                                             start=(kc == 0),
                                             stop=(kc == nkc - 1))
                            nc.tensor.matmul(opsum, vv[:, kc, :], at,
                                             start=(kc == 0),
                                             stop=(kc == nkc - 1))
                        u01 = ap.tile([1, P], F32, tag="u01")
                        nc.vector.tensor_scalar(u01, dsum, 0.0, None,
                                                op0=OP.is_equal)
                        den = ap.tile([1, P], F32, tag="den")
                        nc.vector.tensor_add(den, dsum, u01)
                        rec = ap.tile([1, P], F32R, tag="rec")
                        nc.vector.reciprocal(rec, den)
                        u01s = ap.tile([1, P], F32R, tag="u01s")
                        nc.vector.tensor_scalar_mul(u01s, u01, 1.0 / S)
                        nc.tensor.matmul(opsum, csr, u01s,
                                         start=False, stop=True,
                                         skip_group_check=True)
                        bc_ps = mps.tile([P, P], F32, tag="misc")
                        nc.tensor.matmul(bc_ps, oner_t, rec,
                                         start=True, stop=True)
                        bc = ap.tile([P, P], F32, tag="recbc")
                        nc.scalar.activation(bc, bc_ps, AF.Copy)
                        nc.vector.tensor_mul(
                            aoutT[h][:, qt * P:(qt + 1) * P], opsum, bc)

            # ================= phase D: o_proj =================
            with (
                tc.tile_pool(name="wp2", bufs=4) as wp2,
                tc.tile_pool(name="op2", bufs=3) as op2,
                tc.tile_pool(name="mmps2", bufs=3, space="PSUM") as mmps2,
            ):
                for m in range(HK):
                    ps = mmps2.tile([P, T], F32, tag="mm2")
                    for k in range(NH):
                        wt = wp2.tile([P, P], F32R, tag="wo")
                        nc.sync.dma_start(
                            wt, wo[k * P:(k + 1) * P, m * P:(m + 1) * P])
                        nc.tensor.matmul(ps, wt, aoutT[k],
                                         start=(k == 0), stop=(k == NH - 1))
                    osb = op2.tile([P, T], F32, tag="osb")
                    nc.scalar.activation(osb, ps, AF.Copy)
                    nc.sync.dma_start(outT[m * P:(m + 1) * P, :], osb)

    split_multi_waits(nc)
    return nc


_BUILT = {}


def kernel(**inputs):
    per_core = host_prep(inputs)
    if "nc" not in _BUILT:
        _BUILT["nc"] = build_kernel(with_collective=True)
    nc = _BUILT["nc"]
    res = run_bass_kernel_spmd(nc, per_core, core_ids=list(range(NC)))
    results = [{"outT": r["outT"]} for r in res.results]
    return unshard_output(results)
